# revision 9
# baseline (speedup 1.0000x reference)
"""Trainium2 Bass kernel for out = exp(-M) @ x.

M: [16384, 16384] fp32, x: [16384, 128] fp32 -> out: [16384, 128] fp32.

Sharding: row-shard M and out over 8 cores (2048 rows each), x replicated.

Per-core pipeline (all engines overlapped, DMA-bound at ~128 MiB HBM reads):
  DMA   : M tiles [128, 4096] fp32, natural layout (16 KiB contiguous rows),
          issue alternates SP / ACT sequencers to spread HWDGE setup cost
  ACT   : e = exp(-M_tile) fused fp32 -> bf16 (free affine scale=-1)
  PE    : transpose e chunks [128m, 128k] -> PSUM [128k, 128m] (bf16)
  DVE   : evacuate PSUM -> SBUF rhs tiles [128k, 512m]
  PE    : out.T[f, m] += x[kchunk].T @ rhs   (x stationary bf16, fp32 PSUM acc)
  PE/DVE: final [f, m] -> [m, f] transpose, store via SWDGE
"""

import sys

sys.path.insert(0, "/opt/trn_rl_repo")

import numpy as np

import concourse.bass as bass  # noqa: F401  (engine namespaces live on nc)
import concourse.mybir as mybir
import concourse.tile as tile
from concourse import bacc
from concourse.bass_utils import run_bass_kernel_spmd
from concourse.masks import make_identity

N = 16384  # M is [N, N]
D = 128  # x is [N, D]
N_CORES = 8
M_ROWS = N // N_CORES  # 2048 rows of M / out per core

F32 = mybir.dt.float32
BF16 = mybir.dt.bfloat16
EXP = mybir.ActivationFunctionType.Exp

# geometry
M_SUPER = 512  # output rows accumulated per PSUM bank
N_SUPERS = M_ROWS // M_SUPER  # 4
import os as _os
K_WIN = int(_os.environ.get("KWIN", "4096"))  # contraction window per M DMA tile
N_WINS = N // K_WIN  # 4
M_SUBS = M_SUPER // 128  # 4 m-subtiles per super
KC_PER_WIN = K_WIN // 128  # 32 k-chunks per window
N_KCHUNKS = N // 128  # 128 total k-chunks
X_STAGE = 4096  # x staging chunk (fp32) free-dim


import os

BUFS_M = int(os.environ.get("BUFS_M", "5"))
BUFS_E = int(os.environ.get("BUFS_E", "7"))
SPLIT_DMA = int(os.environ.get("SPLIT_DMA", "1"))
BUFS_PT = int(os.environ.get("BUFS_PT", "5"))
BUFS_RHS = int(os.environ.get("BUFS_RHS", "6"))
KWIN_ENV = int(os.environ.get("KWIN", "4096"))


def build_kernel(repeats=1, mode="full"):
    nc = bacc.Bacc("TRN2", target_bir_lowering=False, debug=False)
    m_ap = nc.dram_tensor("m_shard", [M_ROWS, N], F32, kind="ExternalInput").ap()
    x_ap = nc.dram_tensor("x", [N, D], F32, kind="ExternalInput").ap()
    out_ap = nc.dram_tensor("out", [M_ROWS, D], F32, kind="ExternalOutput").ap()

    from contextlib import ExitStack

    with tile.TileContext(nc) as tc, ExitStack() as ctx:
        consts = ctx.enter_context(tc.tile_pool(name="consts", bufs=1))
        ident_bf = consts.tile([128, 128], BF16)
        make_identity(nc, ident_bf[:])
        ident_f32 = consts.tile([128, 128], F32)
        make_identity(nc, ident_f32[:])

        # x resident in SBUF as bf16, chunk c at xbf[:, c*128:(c+1)*128]
        # (partition = k within chunk, free = feature).  Staged once before
        # the repeat loop: x is the reused operand, M is the streamed one.
        xbf_t = consts.tile([128, N_KCHUNKS * D], BF16)
        if os.environ.get("NOX"):  # NOX=1: sim-only steady-state probe
            nc.vector.memset(xbf_t[:], 0)
        else:
            with tc.tile_pool(name="xstage", bufs=4) as xstage:
                for c in range(N_KCHUNKS):
                    xs = xstage.tile([128, D], F32)
                    x_eng = nc.gpsimd if os.environ.get("X_GPSIMD") else nc.sync
                    x_eng.dma_start(out=xs[:], in_=x_ap[c * 128 : (c + 1) * 128, :])
                    nc.vector.tensor_copy(xbf_t[:, c * D : (c + 1) * D], xs[:])

        if repeats > 1:
            ctx.enter_context(tc.For_i(0, repeats, 1))
        m_pool = ctx.enter_context(tc.tile_pool(name="m", bufs=BUFS_M))
        e_pool = ctx.enter_context(tc.tile_pool(name="e", bufs=BUFS_E))
        rhs_pool = ctx.enter_context(tc.tile_pool(name="rhs", bufs=BUFS_RHS))
        outT_pool = ctx.enter_context(tc.tile_pool(name="outT", bufs=2))
        outf_pool = ctx.enter_context(tc.tile_pool(name="outf", bufs=2))
        pt_pool = ctx.enter_context(tc.tile_pool(name="pt", bufs=BUFS_PT, space="PSUM"))
        pout_pool = ctx.enter_context(tc.tile_pool(name="pout", bufs=2, space="PSUM"))
        pfin_pool = ctx.enter_context(tc.tile_pool(name="pfin", bufs=int(os.environ.get("BUFS_PFIN", "1")), space="PSUM"))

        for ms in range(N_SUPERS):
            pout = (
                pout_pool.tile([128, M_SUPER], F32, name="pout", tag="pout")
                if mode not in ("mem", "dma")
                else None
            )
            outT_mem = (
                outT_pool.tile([128, M_SUPER], F32, name="outT", tag="outT")
                if mode in ("mem", "dma")
                else None
            )
            wins = [(i * K_WIN, K_WIN) for i in range(N_WINS)]
            if ms == N_SUPERS - 1 and int(os.environ.get("TAIL_SPLIT", "1")):
                # shrink the pipeline-drain tail: the very last window is
                # processed in progressively smaller pieces
                lc = (N_WINS - 1) * K_WIN
                wins = wins[:-1] + [
                    (lc, K_WIN // 2),
                    (lc + K_WIN // 2, K_WIN // 4),
                    (lc + 3 * K_WIN // 4, K_WIN // 8),
                    (lc + 7 * K_WIN // 8, K_WIN // 8),
                ]
            for c0, kwidth in wins:
                ebf = []
                for j in range(M_SUBS):
                    mt_full = m_pool.tile([128, K_WIN], F32, name="mt")
                    mt = mt_full[:, :kwidth]
                    r0 = ms * M_SUPER + j * 128
                    w = kwidth // SPLIT_DMA
                    for s in range(SPLIT_DMA):
                        mix = os.environ.get("DMA_MIX", "")
                        idx = j * SPLIT_DMA + s
                        if mix == "hwsw":
                            dma_eng = nc.sync if idx % 2 == 0 else nc.gpsimd
                        elif mix == "3way":
                            dma_eng = (nc.sync, nc.scalar, nc.gpsimd)[idx % 3]
                        elif mix == "sync":
                            dma_eng = nc.sync
                        else:
                            dma_eng = nc.sync if idx % 2 == 0 else nc.scalar
                        dma_eng.dma_start(
                            out=mt[:, s * w : (s + 1) * w],
                            in_=m_ap[r0 : r0 + 128, c0 + s * w : c0 + (s + 1) * w],
                        )
                    if mode == "dma":
                        nc.vector.tensor_copy(
                            outT_mem[:, j * 128 : (j + 1) * 128], mt[:, 0:128]
                        )
                        continue
                    e_full = e_pool.tile([128, K_WIN], BF16, name="e")
                    e = e_full[:, :kwidth]
                    nc.scalar.activation(e[:], mt[:], EXP, scale=-1.0)
                    ebf.append(e)
                if mode == "dma":
                    continue
                if mode == "mem":
                    # probe: DMA + exp only; consume every e tile cheaply
                    for j in range(M_SUBS):
                        nc.vector.tensor_copy(
                            outT_mem[:, j * 128 : (j + 1) * 128], ebf[j][:, 0:128]
                        )
                    continue
                for kc in range(kwidth // 128):
                    kg = c0 // 128 + kc
                    if mode == "noT":
                        # probe: skip transposes+copies; feed MM junk rhs
                        off = max(0, min(kc * 128, kwidth - M_SUPER))
                        nc.tensor.matmul(
                            pout[:],
                            lhsT=xbf_t[:, kg * D : (kg + 1) * D],
                            rhs=ebf[0][:, off : off + M_SUPER],
                            start=(kg == 0),
                            stop=(kg == N_KCHUNKS - 1),
                        )
                        continue
                    pt = pt_pool.tile([128, M_SUPER], BF16)
                    for j in range(M_SUBS):
                        nc.tensor.transpose(
                            pt[:, j * 128 : (j + 1) * 128],
                            ebf[j][:, kc * 128 : (kc + 1) * 128],
                            ident_bf[:],
                        )
                    rhs = rhs_pool.tile([128, M_SUPER], BF16)
                    nc.vector.tensor_copy(rhs[:], pt[:])
                    nc.tensor.matmul(
                        pout[:],
                        lhsT=xbf_t[:, kg * D : (kg + 1) * D],
                        rhs=rhs[:],
                        start=(kg == 0),
                        stop=(kg == N_KCHUNKS - 1),
                    )
            # evacuate out.T [f, m] and transpose to [m, f]
            if mode in ("mem", "dma"):
                outT = outT_mem
            else:
                outT = outT_pool.tile([128, M_SUPER], F32)
                nc.vector.tensor_copy(outT[:], pout[:])
            for j in range(M_SUBS):
                pf = pfin_pool.tile([128, D], F32)
                nc.tensor.transpose(
                    pf[:], outT[:, j * 128 : (j + 1) * 128], ident_f32[:]
                )
                of = outf_pool.tile([128, D], F32)
                nc.vector.tensor_copy(of[:], pf[:])
                r0 = ms * M_SUPER + j * 128
                o_eng = nc.gpsimd if os.environ.get("X_GPSIMD") else nc.scalar
                o_eng.dma_start(out=out_ap[r0 : r0 + 128, :], in_=of[:])

    nc.compile()
    return nc


_NC_CACHE = None


def _get_nc():
    global _NC_CACHE
    if _NC_CACHE is None:
        _NC_CACHE = build_kernel()
    return _NC_CACHE


def _run_on_device(M, x):
    nc = _get_nc()
    in_maps = [
        {"m_shard": M[c * M_ROWS : (c + 1) * M_ROWS], "x": x} for c in range(N_CORES)
    ]
    res = run_bass_kernel_spmd(nc, in_maps, list(range(N_CORES)))
    return np.concatenate([res.results[c]["out"] for c in range(N_CORES)], axis=0)


def _run_in_subprocess(M, x):
    """Retry path: a fresh process gets a fresh NRT/axon session, which
    recovers from the occasional NRT_EXEC_UNIT_UNRECOVERABLE flake."""
    import os, subprocess, tempfile

    d = tempfile.mkdtemp(prefix="bassk_")
    np.save(os.path.join(d, "M.npy"), M)
    np.save(os.path.join(d, "x.npy"), x)
    here = os.path.dirname(os.path.abspath(__file__))
    code = (
        "import sys, numpy as np\n"
        f"sys.path.insert(0, {here!r})\n"
        "import kernel\n"
        f"M = np.load({os.path.join(d, 'M.npy')!r})\n"
        f"x = np.load({os.path.join(d, 'x.npy')!r})\n"
        "out = kernel._run_on_device(M, x)\n"
        f"np.save({os.path.join(d, 'out.npy')!r}, out)\n"
    )
    subprocess.run([sys.executable, "-c", code], check=True, timeout=1200)
    return np.load(os.path.join(d, "out.npy"))


def kernel(M, x):
    M = np.ascontiguousarray(np.asarray(M, dtype=np.float32))
    x = np.ascontiguousarray(np.asarray(x, dtype=np.float32))
    assert M.shape == (N, N) and x.shape == (N, D)
    try:
        return _run_on_device(M, x)
    except Exception as e:
        print(f"kernel: in-process run failed ({e!r}); retrying in subprocess",
              file=sys.stderr, flush=True)
    last = None
    for _ in range(2):
        try:
            return _run_in_subprocess(M, x)
        except Exception as e:  # noqa: PERF203
            last = e
    raise last

